# revision 17
# baseline (speedup 1.0000x reference)
"""Trainium2 Bass kernel for nn_CrossAttention (8-core data-parallel over batch).

Reference math (per batch b, chunk c):
  en = LayerNorm(e) ; q = en@Wq+bq ; k = h@Wk+bk ; v = h@Wv+bv
  attn = softmax(q@k^T / 8) ; o = attn@v ; out = o@Wo + bo + e

Host-side folding:  q = ((e-mu)*rstd) @ (ln_g[:,None]*Wq/8) + (ln_b@Wq+bq)/8
so the on-chip LN is just (e-mu)*rstd.  bv is folded into bo
(softmax rows sum to 1):  boc = bo + bv@Wo.

Single-core program (one batch, 32 chunks), tick-pipelined so the PE never
starves.  Tick t emits, in per-engine critical order:
  PE : xTT-a(t) | scores(t-1)+bden/AV(t-1) | xTT-bc(t) | group-slice PE |
       Qproj(t) | Oproj(t-1) | backT(t-1)
  ACT: exp(t-1) x3 | qT-evac(t) x6 | fT-evac(t-1) x6
  DVE: recip(t-1)/oT-mult(t-1) x6 | resid(t-1) | stats(t+1) | [kbd/v2 evac]
  Pool: xT-evac(t) x3 | vbd(t) | LN-apply(t+1) | [memsets]
plus e-load DMA for chunk t+2 and 1/4 of the next group's h/K/V work
(slice 0: h DMA, 1: h transposes, 2: K-proj, 3: V-proj).

PE-cycle reductions vs the previous version:
  - all PE transposes stream a bf16 identity (cost keys on the moving
    operand: 1.0 cycles/row instead of fp32r's 1.5; data dtype unchanged)
  - softmax denominator is computed directly in dk-broadcast layout with a
    block-diag ones stationary (one matmul per head-pair), removing the
    separate selector-denominator and reciprocal-broadcast matmuls
  - bv folded into boc removes the V-proj bias piggyback matmuls
  - scores/bden/AV run fully in bf16 (q/k/exp/v quantized; well inside the
    2e-2 tolerance)
"""

import numpy as np

B, C, N, S, D = 8, 32, 4, 64, 768
NH, DK = 12, 64
R = N * S          # 256 rows per chunk
KO = D // 128      # 6 partition blocks of d
NP = 6             # head pairs
LN_EPS = 1e-5
GROUP = 4          # chunks per h/kv batch group

_prog_cache = {}


def _build(n_chunks):
    import concourse.bass as bass
    import concourse.tile as tile
    from concourse import bacc, mybir
    from contextlib import ExitStack

    F32 = mybir.dt.float32
    F32R = mybir.dt.float32r
    BF16 = mybir.dt.bfloat16
    I32 = mybir.dt.int32
    AF = mybir.ActivationFunctionType
    ALU = mybir.AluOpType

    nc = bacc.Bacc()
    assert n_chunks % GROUP == 0
    n_groups = n_chunks // GROUP

    d_e = nc.dram_tensor("e", [n_chunks, R, D], F32, kind="ExternalInput")
    d_h = nc.dram_tensor("hbf", [n_chunks, S, D], BF16, kind="ExternalInput")
    d_wq = nc.dram_tensor("wq", [KO, 128, D], BF16, kind="ExternalInput")
    d_wk = nc.dram_tensor("wk", [KO, 128, D], BF16, kind="ExternalInput")
    d_wv = nc.dram_tensor("wv", [KO, 128, D], BF16, kind="ExternalInput")
    d_wo = nc.dram_tensor("wo", [KO, 128, D], BF16, kind="ExternalInput")
    d_bqc = nc.dram_tensor("bqc", [128, KO], F32, kind="ExternalInput")
    d_bkc = nc.dram_tensor("bkc", [128, KO], F32, kind="ExternalInput")
    d_boc = nc.dram_tensor("boc", [128, KO], F32, kind="ExternalInput")
    d_id = nc.dram_tensor("ident", [128, 128], F32, kind="ExternalInput")
    d_out = nc.dram_tensor("out", [n_chunks, R, D], F32, kind="ExternalOutput")

    with ExitStack() as ctx:
        tc = ctx.enter_context(tile.TileContext(nc))
        consts = ctx.enter_context(tc.tile_pool(name="consts", bufs=1))
        e_pool = ctx.enter_context(tc.tile_pool(name="e_pool", bufs=4))
        h2_pool = ctx.enter_context(tc.tile_pool(name="h2_pool", bufs=1))
        x_pool = ctx.enter_context(tc.tile_pool(name="x_pool", bufs=2))
        xT_pool = ctx.enter_context(tc.tile_pool(name="xT_pool", bufs=2))
        q_pool = ctx.enter_context(tc.tile_pool(name="q_pool", bufs=2))
        exp_pool = ctx.enter_context(tc.tile_pool(name="exp_pool", bufs=2))
        oT_pool = ctx.enter_context(tc.tile_pool(name="oT_pool", bufs=1))
        fT_pool = ctx.enter_context(tc.tile_pool(name="fT_pool", bufs=1))
        vd_pool = ctx.enter_context(tc.tile_pool(name="vd_pool", bufs=2))
        rb_pool = ctx.enter_context(tc.tile_pool(name="rb_pool", bufs=3))
        hT_pool = ctx.enter_context(tc.tile_pool(name="hT_pool", bufs=1))
        kt_pool = ctx.enter_context(tc.tile_pool(name="kt_pool", bufs=2))
        v2_pool = ctx.enter_context(tc.tile_pool(name="v2_pool", bufs=2))
        st_pool = ctx.enter_context(tc.tile_pool(name="st_pool", bufs=2))
        ps_x = ctx.enter_context(tc.tile_pool(name="ps_x", bufs=3, space="PSUM"))
        ps_qo = ctx.enter_context(tc.tile_pool(name="ps_qo", bufs=2, space="PSUM"))
        ps_ab = ctx.enter_context(tc.tile_pool(name="ps_ab", bufs=3, space="PSUM"))

        # ---- constants ----
        wq = consts.tile([128, KO, D], BF16)
        wk = consts.tile([128, KO, D], BF16)
        wv = consts.tile([128, KO, D], BF16)
        wo = consts.tile([128, KO, D], BF16)
        nc.sync.dma_start(wk[:], d_wk[:].rearrange("k p d -> p k d"))
        bqc = consts.tile([128, KO], F32)
        bkc = consts.tile([128, KO], F32)
        boc = consts.tile([128, KO], F32)
        nc.sync.dma_start(bqc[:], d_bqc[:])
        nc.sync.dma_start(bkc[:], d_bkc[:])
        nc.sync.dma_start(boc[:], d_boc[:])
        identf = consts.tile([128, 128], F32)
        nc.sync.dma_start(identf[:], d_id[:])
        ident = consts.tile([128, 128], BF16)
        nc.gpsimd.tensor_copy(ident[:], identf[:])
        # block-diag ones [128,128] bf16 (softmax-denominator broadcast)
        obk = consts.tile([128, 128], BF16)
        nc.gpsimd.memset(obk[:], 0.0)
        nc.gpsimd.memset(obk[0:64, 0:64], 1.0)
        nc.gpsimd.memset(obk[64:128, 64:128], 1.0)
        eps_t = consts.tile([128, 1], F32)
        nc.vector.memset(eps_t[:], LN_EPS)

        # ---------- per-phase emission helpers ----------

        def emit_e_load(c):
            e_sb = e_pool.tile([128, 2, D], F32, tag="e")
            nc.sync.dma_start(
                e_sb[:], d_e[c].rearrange("(t p) d -> p t d", p=128))
            return e_sb

        def emit_ln(c, e_sb):
            # LayerNorm stats + rsqrt(var+eps) via bit-hack + 2 Newton steps
            # (DVE only), apply on Pool -> x_sb (f32r)
            stats = st_pool.tile([128, 2, 3, 6], F32, tag="stats")
            mv = st_pool.tile([128, 2, 2], F32, tag="mv")
            rstd = st_pool.tile([128, 2], F32, tag="rstd")
            x_sb = x_pool.tile([128, 2, D], BF16, tag="x")
            for t in range(2):
                esl = e_sb[:, t, :].rearrange("p (s f) -> p s f", s=3)
                for sg in range(3):
                    nc.vector.bn_stats(stats[:, t, sg, :], esl[:, sg, :])
                nc.vector.bn_aggr(mv[:, t, :], stats[:, t, :, :])
            v1 = st_pool.tile([128, 2], F32, tag="v1")
            y = st_pool.tile([128, 2], F32, tag="y")
            tmp = st_pool.tile([128, 2], F32, tag="tmp")
            nc.vector.tensor_scalar(
                out=v1[:], in0=mv[:, :, 1], scalar1=float(LN_EPS), scalar2=None,
                op0=ALU.add)
            nc.vector.tensor_scalar(
                out=y[:].bitcast(I32), in0=v1[:].bitcast(I32), scalar1=1,
                scalar2=None, op0=ALU.logical_shift_right)
            nc.vector.tensor_scalar(
                out=y[:].bitcast(I32), in0=y[:].bitcast(I32), scalar1=-1,
                scalar2=0x5F3759DF, op0=ALU.mult, op1=ALU.add)
            for _ in range(2):
                nc.vector.tensor_tensor(
                    out=tmp[:], in0=y[:], in1=y[:], op=ALU.mult)
                nc.vector.tensor_tensor(
                    out=tmp[:], in0=tmp[:], in1=v1[:], op=ALU.mult)
                nc.vector.tensor_scalar(
                    out=tmp[:], in0=tmp[:], scalar1=-0.5, scalar2=1.5,
                    op0=ALU.mult, op1=ALU.add)
                nc.vector.tensor_tensor(
                    out=rstd[:], in0=y[:], in1=tmp[:], op=ALU.mult)
                nc.vector.tensor_copy(y[:], rstd[:])
            for t in range(2):
                nc.gpsimd.tensor_scalar(
                    out=x_sb[:, t, :], in0=e_sb[:, t, :],
                    scalar1=mv[:, t, 0:1], scalar2=rstd[:, t:t + 1],
                    op0=ALU.subtract, op1=ALU.mult)
            return x_sb

        def emit_xT_dma(x_sb):
            xT = xT_pool.tile([128, KO, R], BF16, tag="xT")
            for ko2 in range(3):
                pt4 = ps_x.tile([128, 4, 128], BF16, tag="x", name="pt4")
                for i in range(2):
                    for t in range(2):
                        nc.tensor.transpose(
                            pt4[:, 2 * i + t, :],
                            x_sb[:, t, (2 * ko2 + i) * 128:(2 * ko2 + i + 1) * 128],
                            ident[:])
                if ko2 < 2:
                    nc.vector.tensor_copy(
                        xT[:, 2 * ko2:2 * ko2 + 2, :], pt4[:])
                else:
                    nc.scalar.copy(xT[:, 2 * ko2:2 * ko2 + 2, :], pt4[:])
            return xT

        def emit_qproj(xT):
            qT = q_pool.tile([128, KO, R], BF16, tag="qT")
            for mo in range(KO):
                pq = ps_qo.tile([128, 512], F32, tag="qo", name="pq")
                for ko in range(KO):
                    nc.tensor.matmul(
                        pq[:, 0:R], wq[:, ko, mo * 128:(mo + 1) * 128],
                        xT[:, ko, :], start=(ko == 0), stop=(ko == KO - 1))
                nc.scalar.activation(
                    qT[:, mo, :], pq[:, 0:R], AF.Identity,
                    bias=bqc[:, mo:mo + 1], scale=1.0)
            return qT

        def emit_vbd(cc, v2):
            # v in block-diagonal head-pair layout (bf16, Pool)
            v2t = v2[cc // 2]
            pa = 64 * (cc % 2)
            vbd = vd_pool.tile([128, NP, 128], BF16, tag="vbd")
            nc.gpsimd.memset(vbd[:], 0.0)
            v2v = v2t[pa:pa + 64, :].rearrange(
                "p (np two dk) -> p np two dk", np=NP, two=2)
            nc.gpsimd.tensor_copy(vbd[0:64, :, 0:DK], v2v[:, :, 0, :])
            nc.gpsimd.tensor_copy(vbd[64:128, :, DK:128], v2v[:, :, 1, :])
            return vbd

        def emit_scores(cc, qT, kbd):
            expT = exp_pool.tile([128, NP, R], BF16, tag="expT")
            for p2 in range(0, NP, 2):
                pscr = ps_x.tile([128, 2, R], F32, tag="x", name="pscr")
                for i in range(2):
                    nc.tensor.matmul(
                        pscr[:, i, :], kbd[:, p2 + i, cc, :], qT[:, p2 + i, :],
                        start=True, stop=True)
                nc.scalar.activation(
                    expT[:, p2:p2 + 2, :], pscr[:], AF.Exp, bias=0.0, scale=1.0)
            return expT

        def emit_attn_q(expT, vbd, xT):
            # attention pairs interleaved with Q-proj mo-blocks: the Q matmuls
            # are independent PE filler that covers the DVE recip/mult latency
            # gating the ps_ab rotation
            oT = oT_pool.tile([128, KO, R], BF16, tag="oT")
            qT = None
            if xT is not None:
                qT = q_pool.tile([128, KO, R], BF16, tag="qT")
            q_sched = {0: [0], 1: [1], 2: [2, 3, 4, 5]}

            def q_mo(mo):
                pq = ps_qo.tile([128, 512], F32, tag="qo", name="pq")
                for ko in range(KO):
                    nc.tensor.matmul(
                        pq[:, 0:R], wq[:, ko, mo * 128:(mo + 1) * 128],
                        xT[:, ko, :], start=(ko == 0), stop=(ko == KO - 1))
                nc.scalar.activation(
                    qT[:, mo, :], pq[:, 0:R], AF.Identity,
                    bias=bqc[:, mo:mo + 1], scale=1.0)

            for pi, p2 in enumerate(range(0, NP, 2)):
                pbd = ps_ab.tile([128, 2, R], F32, tag="ab", name="pbd")
                for i in range(2):
                    nc.tensor.matmul(
                        pbd[:, i, :], obk[:], expT[:, p2 + i, :],
                        start=True, stop=True)
                pav = ps_ab.tile([128, 2, R], F32, tag="ab", name="pav")
                for i in range(2):
                    nc.tensor.matmul(
                        pav[:, i, :], vbd[:, p2 + i, :], expT[:, p2 + i, :],
                        start=True, stop=True)
                rbd = rb_pool.tile([128, 2, R], F32, tag="rbd")
                with nc.allow_low_precision(reason="softmax denom"):
                    nc.vector.reciprocal(rbd[:], pbd[:])
                nc.vector.tensor_tensor(
                    out=oT[:, p2:p2 + 2, :], in0=pav[:], in1=rbd[:],
                    op=ALU.mult)
                if xT is not None:
                    for mo in q_sched[pi]:
                        q_mo(mo)
            return oT, qT

        def emit_oproj(oT):
            fT = fT_pool.tile([128, KO, R], BF16, tag="fT")
            for mo in range(KO):
                pf = ps_qo.tile([128, 512], F32, tag="qo", name="pf")
                for ko in range(KO):
                    nc.tensor.matmul(
                        pf[:, 0:R], wo[:, ko, mo * 128:(mo + 1) * 128],
                        oT[:, ko, :], start=(ko == 0), stop=(ko == KO - 1))
                nc.scalar.activation(
                    fT[:, mo, :], pf[:, 0:R], AF.Identity,
                    bias=boc[:, mo:mo + 1], scale=1.0)
            return fT

        def emit_backT_store(c, fT, e_sb):
            for t in range(2):
                for m0, mn in ((0, 4), (4, 2)):
                    ptq = ps_x.tile([128, 4, 128], BF16, tag="x", name="ptq")
                    for i in range(mn):
                        nc.tensor.transpose(
                            ptq[:, i, :], fT[:, m0 + i, t * 128:(t + 1) * 128],
                            ident[:])
                    nc.vector.tensor_tensor(
                        out=e_sb[:, t, m0 * 128:(m0 + mn) * 128],
                        in0=ptq[:, 0:mn, :],
                        in1=e_sb[:, t, m0 * 128:(m0 + mn) * 128],
                        op=ALU.add)
            nc.sync.dma_start(
                d_out[c].rearrange("(t p) d -> p t d", p=128), e_sb[:])

        # ---------- group-phase slices ----------

        def emit_h_load(g):
            h2b = h2_pool.tile([S, GROUP, D], BF16, tag="h2b")
            nc.sync.dma_start(
                h2b[:], d_h[g * GROUP:(g + 1) * GROUP].rearrange("c j d -> j c d"))
            return h2b

        def emit_hT(h2b):
            hT4 = hT_pool.tile([128, KO, GROUP * S], BF16, tag="hT4")
            for cc in range(GROUP):
                for k0, kn in ((0, 4), (4, 2)):
                    ptq = ps_x.tile([128, 4, 128], BF16, tag="x", name="pth")
                    for i in range(kn):
                        nc.tensor.transpose(
                            ptq[:, i, 0:S],
                            h2b[:, cc, (k0 + i) * 128:(k0 + i + 1) * 128],
                            ident[0:S, 0:S])
                    nc.vector.tensor_copy(
                        hT4[:, k0:k0 + kn, cc * S:(cc + 1) * S],
                        ptq[:, 0:kn, 0:S])
            return hT4

        def emit_kproj(hT4):
            # kT in block-diagonal pair layout (bf16)
            kbd = kt_pool.tile([128, NP, GROUP, 128], BF16, tag="kbd")
            nc.gpsimd.memset(kbd[:], 0.0)
            for mo in range(KO):
                pk = ps_qo.tile([128, 512], F32, tag="qo", name="pk")
                for ko in range(KO):
                    nc.tensor.matmul(
                        pk[:, 0:GROUP * S], wk[:, ko, mo * 128:(mo + 1) * 128],
                        hT4[:, ko, :], start=(ko == 0), stop=(ko == KO - 1))
                pkv = pk[:, 0:GROUP * S].rearrange("p (c j) -> p c j", c=GROUP)
                nc.scalar.activation(
                    kbd[0:64, mo, :, 0:S], pkv[0:64], AF.Identity,
                    bias=bkc[0:64, mo:mo + 1], scale=1.0)
                nc.scalar.activation(
                    kbd[64:128, mo, :, S:128], pkv[64:128], AF.Identity,
                    bias=bkc[64:128, mo:mo + 1], scale=1.0)
            return kbd

        def emit_vproj(hT4):
            v2 = []
            for st in range(GROUP // 2):
                v2t = v2_pool.tile([128, D], BF16, tag=f"v2{st}")
                for n0, ns in ((0, 512), (512, 256)):
                    pv = ps_qo.tile([128, 512], F32, tag="qo", name="pv")
                    for ko in range(KO):
                        nc.tensor.matmul(
                            pv[:, 0:ns],
                            hT4[:, ko, st * 128:(st + 1) * 128],
                            wv[:, ko, n0:n0 + ns],
                            start=(ko == 0), stop=(ko == KO - 1))
                    nc.scalar.copy(v2t[:, n0:n0 + ns], pv[:, 0:ns])
                v2.append(v2t)
            return v2

        # ---------- driver ----------

        # chunk-state carried across ticks
        e_sb_of = {}
        x_of = {}
        xT_of = {}
        qT_of = {}
        vbd_of = {}
        # group-state
        grp = {}   # g -> dict(h2, hT4, kbd, v2)

        # prologue: group 0 fully, e(0), e(1), LN(0)
        grp[0] = {}
        h2b0 = emit_h_load(0)
        nc.sync.dma_start(wv[:], d_wv[:].rearrange("k p d -> p k d"))
        grp[0]["hT4"] = emit_hT(h2b0)
        e_sb_of[0] = emit_e_load(0)
        if n_chunks > 1:
            e_sb_of[1] = emit_e_load(1)
        grp[0]["kbd"] = emit_kproj(grp[0]["hT4"])
        nc.sync.dma_start(wq[:], d_wq[:].rearrange("k p d -> p k d"))
        grp[0]["v2"] = emit_vproj(grp[0]["hT4"])
        nc.sync.dma_start(wo[:], d_wo[:].rearrange("k p d -> p k d"))
        x_of[0] = emit_ln(0, e_sb_of[0])

        for t in range(n_chunks + 1):
            cb, ca, cp = t - 1, t, t + 1
            if t + 2 < n_chunks:
                e_sb_of[t + 2] = emit_e_load(t + 2)

            # B-phase scores first (all inputs ready; starts the ACT exp
            # pipeline), then A-phase transposes fill PE while exp completes
            if cb >= 0:
                g_b = cb // GROUP
                expT = emit_scores(cb % GROUP, qT_of.pop(cb), grp[g_b]["kbd"])
            if ca < n_chunks:
                xT_of[ca] = emit_xT_dma(x_of.pop(ca))
            if cb >= 0:
                oT, qTn = emit_attn_q(
                    expT, vbd_of.pop(cb),
                    xT_of.pop(ca) if ca < n_chunks else None)
                if qTn is not None:
                    qT_of[ca] = qTn
            elif ca < n_chunks:
                qT_of[ca] = emit_qproj(xT_of.pop(ca))

            # group-slice PE work (ready filler between AV and Q/O)
            gn = t // GROUP + 1
            sl = t % GROUP
            if gn < n_groups:
                if sl == 0:
                    grp[gn] = {"h2b": emit_h_load(gn)}
                elif sl == 1:
                    grp[gn]["hT4"] = emit_hT(grp[gn].pop("h2b"))
                elif sl == 2:
                    grp[gn]["kbd"] = emit_kproj(grp[gn]["hT4"])
                elif sl == 3:
                    grp[gn]["v2"] = emit_vproj(grp[gn]["hT4"])
                    grp.pop(gn - 2, None)

            if ca < n_chunks:
                vbd_of[ca] = emit_vbd(ca % GROUP, grp[ca // GROUP]["v2"])

            if cb >= 0:
                fT = emit_oproj(oT)
                emit_backT_store(cb, fT, e_sb_of.pop(cb))

            if cp < n_chunks:
                x_of[cp] = emit_ln(cp, e_sb_of[cp])

    nc.compile()
    return nc


def _prep_consts(Wq, bq, Wk, bk, Wv, bv, Wo, bo, ln_g, ln_b):
    scale = 1.0 / np.sqrt(DK)
    Wq_eff = (ln_g[:, None] * Wq) * scale
    bq_eff = (ln_b @ Wq + bq) * scale
    bo_eff = bo + bv @ Wo   # softmax rows sum to 1

    import ml_dtypes

    def wl(w):
        return np.ascontiguousarray(
            np.asarray(w, np.float32).reshape(KO, 128, D)).astype(
                ml_dtypes.bfloat16)

    return {
        "wq": wl(Wq_eff), "wk": wl(Wk), "wv": wl(Wv), "wo": wl(Wo),
        "bqc": np.ascontiguousarray(bq_eff.reshape(KO, 128).T, dtype=np.float32),
        "bkc": np.ascontiguousarray(bk.reshape(KO, 128).T, dtype=np.float32),
        "boc": np.ascontiguousarray(bo_eff.reshape(KO, 128).T, dtype=np.float32),
        "ident": np.eye(128, dtype=np.float32),
    }


def kernel(e, h, Wq, bq, Wk, bk, Wv, bv, Wo, bo, ln_g, ln_b):
    from concourse.bass_utils import run_bass_kernel_spmd

    e = np.asarray(e, dtype=np.float32)
    h = np.asarray(h, dtype=np.float32)
    n_chunks = e.shape[1]

    if n_chunks not in _prog_cache:
        _prog_cache[n_chunks] = _build(n_chunks)
    nc = _prog_cache[n_chunks]

    consts = _prep_consts(
        np.asarray(Wq, np.float32), np.asarray(bq, np.float32),
        np.asarray(Wk, np.float32), np.asarray(bk, np.float32),
        np.asarray(Wv, np.float32), np.asarray(bv, np.float32),
        np.asarray(Wo, np.float32), np.asarray(bo, np.float32),
        np.asarray(ln_g, np.float32), np.asarray(ln_b, np.float32))

    in_maps = []
    for b in range(B):
        m = dict(consts)
        import ml_dtypes
        m["e"] = np.ascontiguousarray(e[b].reshape(n_chunks, R, D))
        m["hbf"] = np.ascontiguousarray(h[b]).astype(ml_dtypes.bfloat16)
        in_maps.append(m)

    res = run_bass_kernel_spmd(nc, in_maps, core_ids=list(range(B)))
    out = np.stack([r["out"] for r in res.results], axis=0)
    return out.reshape(B, n_chunks, N, S, D)


# revision 18
# speedup vs baseline: 1.0500x; 1.0500x over previous
"""Trainium2 Bass kernel for nn_CrossAttention (8-core data-parallel over batch).

Reference math (per batch b, chunk c):
  en = LayerNorm(e) ; q = en@Wq+bq ; k = h@Wk+bk ; v = h@Wv+bv
  attn = softmax(q@k^T / 8) ; o = attn@v ; out = o@Wo + bo + e

Host-side folding:  q = ((e-mu)*rstd) @ (ln_g[:,None]*Wq/8) + (ln_b@Wq+bq)/8
so the on-chip LN is just (e-mu)*rstd.  bv is folded into bo
(softmax rows sum to 1):  boc = bo + bv@Wo.

Single-core program (one batch, 32 chunks), tick-pipelined so the PE never
starves.  Tick t emits, in per-engine critical order:
  PE : xTT-a(t) | scores(t-1)+bden/AV(t-1) | xTT-bc(t) | group-slice PE |
       Qproj(t) | Oproj(t-1) | backT(t-1)
  ACT: exp(t-1) x3 | qT-evac(t) x6 | fT-evac(t-1) x6
  DVE: recip(t-1)/oT-mult(t-1) x6 | resid(t-1) | stats(t+1) | [kbd/v2 evac]
  Pool: xT-evac(t) x3 | vbd(t) | LN-apply(t+1) | [memsets]
plus e-load DMA for chunk t+2 and 1/4 of the next group's h/K/V work
(slice 0: h DMA, 1: h transposes, 2: K-proj, 3: V-proj).

PE-cycle reductions vs the previous version:
  - all PE transposes stream a bf16 identity (cost keys on the moving
    operand: 1.0 cycles/row instead of fp32r's 1.5; data dtype unchanged)
  - softmax denominator is computed directly in dk-broadcast layout with a
    block-diag ones stationary (one matmul per head-pair), removing the
    separate selector-denominator and reciprocal-broadcast matmuls
  - bv folded into boc removes the V-proj bias piggyback matmuls
  - scores/bden/AV run fully in bf16 (q/k/exp/v quantized; well inside the
    2e-2 tolerance)
"""

import numpy as np

B, C, N, S, D = 8, 32, 4, 64, 768
NH, DK = 12, 64
R = N * S          # 256 rows per chunk
KO = D // 128      # 6 partition blocks of d
NP = 6             # head pairs
LN_EPS = 1e-5
GROUP = 4          # chunks per h/kv batch group

_prog_cache = {}


def _build(n_chunks):
    import concourse.bass as bass
    import concourse.tile as tile
    from concourse import bacc, mybir
    from contextlib import ExitStack

    F32 = mybir.dt.float32
    F32R = mybir.dt.float32r
    BF16 = mybir.dt.bfloat16
    I32 = mybir.dt.int32
    AF = mybir.ActivationFunctionType
    ALU = mybir.AluOpType

    nc = bacc.Bacc()
    assert n_chunks % GROUP == 0
    n_groups = n_chunks // GROUP

    d_e = nc.dram_tensor("e", [n_chunks, R, D], F32, kind="ExternalInput")
    d_h = nc.dram_tensor("hbf", [n_chunks, S, D], BF16, kind="ExternalInput")
    d_wq = nc.dram_tensor("wq", [KO, 128, D], BF16, kind="ExternalInput")
    d_wk = nc.dram_tensor("wk", [KO, 128, D], BF16, kind="ExternalInput")
    d_wv = nc.dram_tensor("wv", [KO, 128, D], BF16, kind="ExternalInput")
    d_wo = nc.dram_tensor("wo", [KO, 128, D], BF16, kind="ExternalInput")
    d_bqc = nc.dram_tensor("bqc", [128, KO], F32, kind="ExternalInput")
    d_bkc = nc.dram_tensor("bkc", [128, KO], F32, kind="ExternalInput")
    d_boc = nc.dram_tensor("boc", [128, KO], F32, kind="ExternalInput")
    d_id = nc.dram_tensor("ident", [128, 128], F32, kind="ExternalInput")
    d_out = nc.dram_tensor("out", [n_chunks, R, D], F32, kind="ExternalOutput")

    with ExitStack() as ctx:
        tc = ctx.enter_context(tile.TileContext(nc))
        consts = ctx.enter_context(tc.tile_pool(name="consts", bufs=1))
        e_pool = ctx.enter_context(tc.tile_pool(name="e_pool", bufs=4))
        h2_pool = ctx.enter_context(tc.tile_pool(name="h2_pool", bufs=1))
        x_pool = ctx.enter_context(tc.tile_pool(name="x_pool", bufs=2))
        xT_pool = ctx.enter_context(tc.tile_pool(name="xT_pool", bufs=2))
        q_pool = ctx.enter_context(tc.tile_pool(name="q_pool", bufs=2))
        exp_pool = ctx.enter_context(tc.tile_pool(name="exp_pool", bufs=2))
        oT_pool = ctx.enter_context(tc.tile_pool(name="oT_pool", bufs=1))
        fT_pool = ctx.enter_context(tc.tile_pool(name="fT_pool", bufs=1))
        vd_pool = ctx.enter_context(tc.tile_pool(name="vd_pool", bufs=2))
        rb_pool = ctx.enter_context(tc.tile_pool(name="rb_pool", bufs=3))
        hT_pool = ctx.enter_context(tc.tile_pool(name="hT_pool", bufs=1))
        kt_pool = ctx.enter_context(tc.tile_pool(name="kt_pool", bufs=2))
        v2_pool = ctx.enter_context(tc.tile_pool(name="v2_pool", bufs=2))
        st_pool = ctx.enter_context(tc.tile_pool(name="st_pool", bufs=2))
        ps_x = ctx.enter_context(tc.tile_pool(name="ps_x", bufs=3, space="PSUM"))
        ps_qo = ctx.enter_context(tc.tile_pool(name="ps_qo", bufs=2, space="PSUM"))
        ps_ab = ctx.enter_context(tc.tile_pool(name="ps_ab", bufs=3, space="PSUM"))

        # ---- constants ----
        wq = consts.tile([128, KO, D], BF16)
        wk = consts.tile([128, KO, D], BF16)
        wv = consts.tile([128, KO, D], BF16)
        wo = consts.tile([128, KO, D], BF16)
        nc.sync.dma_start(wk[:], d_wk[:].rearrange("k p d -> p k d"))
        bqc = consts.tile([128, KO], F32)
        bkc = consts.tile([128, KO], F32)
        boc = consts.tile([128, KO], F32)
        nc.sync.dma_start(bqc[:], d_bqc[:])
        nc.sync.dma_start(bkc[:], d_bkc[:])
        nc.sync.dma_start(boc[:], d_boc[:])
        identf = consts.tile([128, 128], F32)
        nc.sync.dma_start(identf[:], d_id[:])
        ident = consts.tile([128, 128], BF16)
        nc.gpsimd.tensor_copy(ident[:], identf[:])
        # block-diag ones [128,128] bf16 (softmax-denominator broadcast)
        obk = consts.tile([128, 128], BF16)
        nc.gpsimd.memset(obk[:], 0.0)
        nc.gpsimd.memset(obk[0:64, 0:64], 1.0)
        nc.gpsimd.memset(obk[64:128, 64:128], 1.0)
        eps_t = consts.tile([128, 1], F32)
        nc.vector.memset(eps_t[:], LN_EPS)

        # ---------- per-phase emission helpers ----------

        def emit_e_load(c):
            e_sb = e_pool.tile([128, 2, D], F32, tag="e")
            nc.sync.dma_start(
                e_sb[:], d_e[c].rearrange("(t p) d -> p t d", p=128))
            return e_sb

        def emit_ln(c, e_sb):
            # LayerNorm stats + rsqrt(var+eps) via bit-hack + 2 Newton steps
            # (DVE only), apply on Pool -> x_sb (f32r)
            stats = st_pool.tile([128, 2, 3, 6], F32, tag="stats")
            mv = st_pool.tile([128, 2, 2], F32, tag="mv")
            rstd = st_pool.tile([128, 2], F32, tag="rstd")
            x_sb = x_pool.tile([128, 2, D], BF16, tag="x")
            for t in range(2):
                esl = e_sb[:, t, :].rearrange("p (s f) -> p s f", s=3)
                for sg in range(3):
                    nc.vector.bn_stats(stats[:, t, sg, :], esl[:, sg, :])
                nc.vector.bn_aggr(mv[:, t, :], stats[:, t, :, :])
            v1 = st_pool.tile([128, 2], F32, tag="v1")
            y = st_pool.tile([128, 2], F32, tag="y")
            tmp = st_pool.tile([128, 2], F32, tag="tmp")
            nc.vector.tensor_scalar(
                out=v1[:], in0=mv[:, :, 1], scalar1=float(LN_EPS), scalar2=None,
                op0=ALU.add)
            nc.vector.tensor_scalar(
                out=y[:].bitcast(I32), in0=v1[:].bitcast(I32), scalar1=1,
                scalar2=None, op0=ALU.logical_shift_right)
            nc.vector.tensor_scalar(
                out=y[:].bitcast(I32), in0=y[:].bitcast(I32), scalar1=-1,
                scalar2=0x5F3759DF, op0=ALU.mult, op1=ALU.add)
            for _ in range(2):
                nc.vector.tensor_tensor(
                    out=tmp[:], in0=y[:], in1=y[:], op=ALU.mult)
                nc.vector.tensor_tensor(
                    out=tmp[:], in0=tmp[:], in1=v1[:], op=ALU.mult)
                nc.vector.tensor_scalar(
                    out=tmp[:], in0=tmp[:], scalar1=-0.5, scalar2=1.5,
                    op0=ALU.mult, op1=ALU.add)
                nc.vector.tensor_tensor(
                    out=rstd[:], in0=y[:], in1=tmp[:], op=ALU.mult)
                nc.vector.tensor_copy(y[:], rstd[:])
            for t in range(2):
                nc.gpsimd.tensor_scalar(
                    out=x_sb[:, t, :], in0=e_sb[:, t, :],
                    scalar1=mv[:, t, 0:1], scalar2=rstd[:, t:t + 1],
                    op0=ALU.subtract, op1=ALU.mult)
            return x_sb

        def emit_xT_dma(x_sb):
            xT = xT_pool.tile([128, KO, R], BF16, tag="xT")
            for ko2 in range(3):
                pt4 = ps_x.tile([128, 4, 128], BF16, tag="x", name="pt4")
                for i in range(2):
                    for t in range(2):
                        nc.tensor.transpose(
                            pt4[:, 2 * i + t, :],
                            x_sb[:, t, (2 * ko2 + i) * 128:(2 * ko2 + i + 1) * 128],
                            ident[:])
                if ko2 < 2:
                    nc.vector.tensor_copy(
                        xT[:, 2 * ko2:2 * ko2 + 2, :], pt4[:])
                else:
                    nc.scalar.copy(xT[:, 2 * ko2:2 * ko2 + 2, :], pt4[:])
            return xT

        def emit_qproj(xT):
            qT = q_pool.tile([128, KO, R], BF16, tag="qT")
            for mo in range(KO):
                pq = ps_qo.tile([128, 512], F32, tag="qo", name="pq")
                for ko in range(KO):
                    nc.tensor.matmul(
                        pq[:, 0:R], wq[:, ko, mo * 128:(mo + 1) * 128],
                        xT[:, ko, :], start=(ko == 0), stop=(ko == KO - 1))
                nc.scalar.activation(
                    qT[:, mo, :], pq[:, 0:R], AF.Identity,
                    bias=bqc[:, mo:mo + 1], scale=1.0)
            return qT

        def emit_vbd(cc, v2):
            # v in block-diagonal head-pair layout (bf16, Pool)
            v2t = v2[cc // 2]
            pa = 64 * (cc % 2)
            vbd = vd_pool.tile([128, NP, 128], BF16, tag="vbd")
            nc.gpsimd.memset(vbd[:], 0.0)
            v2v = v2t[pa:pa + 64, :].rearrange(
                "p (np two dk) -> p np two dk", np=NP, two=2)
            nc.gpsimd.tensor_copy(vbd[0:64, :, 0:DK], v2v[:, :, 0, :])
            nc.gpsimd.tensor_copy(vbd[64:128, :, DK:128], v2v[:, :, 1, :])
            return vbd

        def emit_scores(cc, qT, kbd):
            expT = exp_pool.tile([128, NP, R], BF16, tag="expT")
            for p2 in range(0, NP, 2):
                pscr = ps_x.tile([128, 2, R], F32, tag="x", name="pscr")
                for i in range(2):
                    nc.tensor.matmul(
                        pscr[:, i, :], kbd[:, p2 + i, cc, :], qT[:, p2 + i, :],
                        start=True, stop=True)
                nc.scalar.activation(
                    expT[:, p2:p2 + 2, :], pscr[:], AF.Exp, bias=0.0, scale=1.0)
            return expT

        def emit_attn(expT, vbd):
            # per pair: bden (block-ones matmul -> denom broadcast over dk
            # partitions), AV; DVE: reciprocal + normalize into oT
            oT = oT_pool.tile([128, KO, R], BF16, tag="oT")
            for p2 in range(0, NP, 2):
                pbd = ps_ab.tile([128, 2, R], F32, tag="ab", name="pbd")
                for i in range(2):
                    nc.tensor.matmul(
                        pbd[:, i, :], obk[:], expT[:, p2 + i, :],
                        start=True, stop=True)
                pav = ps_ab.tile([128, 2, R], F32, tag="ab", name="pav")
                for i in range(2):
                    nc.tensor.matmul(
                        pav[:, i, :], vbd[:, p2 + i, :], expT[:, p2 + i, :],
                        start=True, stop=True)
                rbd = rb_pool.tile([128, 2, R], F32, tag="rbd")
                with nc.allow_low_precision(reason="softmax denom"):
                    nc.vector.reciprocal(rbd[:], pbd[:])
                nc.vector.tensor_tensor(
                    out=oT[:, p2:p2 + 2, :], in0=pav[:], in1=rbd[:],
                    op=ALU.mult)
            return oT

        def emit_oproj(oT):
            fT = fT_pool.tile([128, KO, R], BF16, tag="fT")
            for mo in range(KO):
                pf = ps_qo.tile([128, 512], F32, tag="qo", name="pf")
                for ko in range(KO):
                    nc.tensor.matmul(
                        pf[:, 0:R], wo[:, ko, mo * 128:(mo + 1) * 128],
                        oT[:, ko, :], start=(ko == 0), stop=(ko == KO - 1))
                nc.scalar.activation(
                    fT[:, mo, :], pf[:, 0:R], AF.Identity,
                    bias=boc[:, mo:mo + 1], scale=1.0)
            return fT

        def emit_backT_store(c, fT, e_sb):
            for t in range(2):
                for m0, mn in ((0, 4), (4, 2)):
                    ptq = ps_x.tile([128, 4, 128], BF16, tag="x", name="ptq")
                    for i in range(mn):
                        nc.tensor.transpose(
                            ptq[:, i, :], fT[:, m0 + i, t * 128:(t + 1) * 128],
                            ident[:])
                    nc.vector.tensor_tensor(
                        out=e_sb[:, t, m0 * 128:(m0 + mn) * 128],
                        in0=ptq[:, 0:mn, :],
                        in1=e_sb[:, t, m0 * 128:(m0 + mn) * 128],
                        op=ALU.add)
            nc.sync.dma_start(
                d_out[c].rearrange("(t p) d -> p t d", p=128), e_sb[:])

        # ---------- group-phase slices ----------

        def emit_h_load(g):
            h2b = h2_pool.tile([S, GROUP, D], BF16, tag="h2b")
            nc.sync.dma_start(
                h2b[:], d_h[g * GROUP:(g + 1) * GROUP].rearrange("c j d -> j c d"))
            return h2b

        def emit_hT(h2b):
            hT4 = hT_pool.tile([128, KO, GROUP * S], BF16, tag="hT4")
            for cc in range(GROUP):
                for k0, kn in ((0, 4), (4, 2)):
                    ptq = ps_x.tile([128, 4, 128], BF16, tag="x", name="pth")
                    for i in range(kn):
                        nc.tensor.transpose(
                            ptq[:, i, 0:S],
                            h2b[:, cc, (k0 + i) * 128:(k0 + i + 1) * 128],
                            ident[0:S, 0:S])
                    nc.vector.tensor_copy(
                        hT4[:, k0:k0 + kn, cc * S:(cc + 1) * S],
                        ptq[:, 0:kn, 0:S])
            return hT4

        def emit_kproj(hT4):
            # kT in block-diagonal pair layout (bf16)
            kbd = kt_pool.tile([128, NP, GROUP, 128], BF16, tag="kbd")
            nc.gpsimd.memset(kbd[:], 0.0)
            for mo in range(KO):
                pk = ps_qo.tile([128, 512], F32, tag="qo", name="pk")
                for ko in range(KO):
                    nc.tensor.matmul(
                        pk[:, 0:GROUP * S], wk[:, ko, mo * 128:(mo + 1) * 128],
                        hT4[:, ko, :], start=(ko == 0), stop=(ko == KO - 1))
                pkv = pk[:, 0:GROUP * S].rearrange("p (c j) -> p c j", c=GROUP)
                nc.scalar.activation(
                    kbd[0:64, mo, :, 0:S], pkv[0:64], AF.Identity,
                    bias=bkc[0:64, mo:mo + 1], scale=1.0)
                nc.scalar.activation(
                    kbd[64:128, mo, :, S:128], pkv[64:128], AF.Identity,
                    bias=bkc[64:128, mo:mo + 1], scale=1.0)
            return kbd

        def emit_vproj(hT4):
            v2 = []
            for st in range(GROUP // 2):
                v2t = v2_pool.tile([128, D], BF16, tag=f"v2{st}")
                for n0, ns in ((0, 512), (512, 256)):
                    pv = ps_qo.tile([128, 512], F32, tag="qo", name="pv")
                    for ko in range(KO):
                        nc.tensor.matmul(
                            pv[:, 0:ns],
                            hT4[:, ko, st * 128:(st + 1) * 128],
                            wv[:, ko, n0:n0 + ns],
                            start=(ko == 0), stop=(ko == KO - 1))
                    nc.scalar.copy(v2t[:, n0:n0 + ns], pv[:, 0:ns])
                v2.append(v2t)
            return v2

        # ---------- driver ----------

        # chunk-state carried across ticks
        e_sb_of = {}
        x_of = {}
        xT_of = {}
        qT_of = {}
        vbd_of = {}
        # group-state
        grp = {}   # g -> dict(h2, hT4, kbd, v2)

        # prologue: group 0 fully, e(0), e(1), LN(0)
        grp[0] = {}
        h2b0 = emit_h_load(0)
        nc.sync.dma_start(wv[:], d_wv[:].rearrange("k p d -> p k d"))
        grp[0]["hT4"] = emit_hT(h2b0)
        e_sb_of[0] = emit_e_load(0)
        if n_chunks > 1:
            e_sb_of[1] = emit_e_load(1)
        grp[0]["kbd"] = emit_kproj(grp[0]["hT4"])
        nc.sync.dma_start(wq[:], d_wq[:].rearrange("k p d -> p k d"))
        grp[0]["v2"] = emit_vproj(grp[0]["hT4"])
        nc.sync.dma_start(wo[:], d_wo[:].rearrange("k p d -> p k d"))
        x_of[0] = emit_ln(0, e_sb_of[0])

        for t in range(n_chunks + 1):
            cb, ca, cp = t - 1, t, t + 1
            if t + 2 < n_chunks:
                e_sb_of[t + 2] = emit_e_load(t + 2)

            # B-phase scores first (all inputs ready; starts the ACT exp
            # pipeline), then A-phase transposes fill PE while exp completes
            if cb >= 0:
                g_b = cb // GROUP
                expT = emit_scores(cb % GROUP, qT_of.pop(cb), grp[g_b]["kbd"])
            if ca < n_chunks:
                xT_of[ca] = emit_xT_dma(x_of.pop(ca))
            if cb >= 0:
                oT = emit_attn(expT, vbd_of.pop(cb))

            # group-slice PE work (ready filler between AV and Q/O)
            gn = t // GROUP + 1
            sl = t % GROUP
            if gn < n_groups:
                if sl == 0:
                    grp[gn] = {"h2b": emit_h_load(gn)}
                elif sl == 1:
                    grp[gn]["hT4"] = emit_hT(grp[gn].pop("h2b"))
                elif sl == 2:
                    grp[gn]["kbd"] = emit_kproj(grp[gn]["hT4"])
                elif sl == 3:
                    grp[gn]["v2"] = emit_vproj(grp[gn]["hT4"])
                    grp.pop(gn - 2, None)

            if ca < n_chunks:
                qT_of[ca] = emit_qproj(xT_of.pop(ca))
                vbd_of[ca] = emit_vbd(ca % GROUP, grp[ca // GROUP]["v2"])

            if cb >= 0:
                fT = emit_oproj(oT)
                emit_backT_store(cb, fT, e_sb_of.pop(cb))

            if cp < n_chunks:
                x_of[cp] = emit_ln(cp, e_sb_of[cp])

    nc.compile()
    return nc


def _prep_consts(Wq, bq, Wk, bk, Wv, bv, Wo, bo, ln_g, ln_b):
    scale = 1.0 / np.sqrt(DK)
    Wq_eff = (ln_g[:, None] * Wq) * scale
    bq_eff = (ln_b @ Wq + bq) * scale
    bo_eff = bo + bv @ Wo   # softmax rows sum to 1

    import ml_dtypes

    def wl(w):
        return np.ascontiguousarray(
            np.asarray(w, np.float32).reshape(KO, 128, D)).astype(
                ml_dtypes.bfloat16)

    return {
        "wq": wl(Wq_eff), "wk": wl(Wk), "wv": wl(Wv), "wo": wl(Wo),
        "bqc": np.ascontiguousarray(bq_eff.reshape(KO, 128).T, dtype=np.float32),
        "bkc": np.ascontiguousarray(bk.reshape(KO, 128).T, dtype=np.float32),
        "boc": np.ascontiguousarray(bo_eff.reshape(KO, 128).T, dtype=np.float32),
        "ident": np.eye(128, dtype=np.float32),
    }


def kernel(e, h, Wq, bq, Wk, bk, Wv, bv, Wo, bo, ln_g, ln_b):
    from concourse.bass_utils import run_bass_kernel_spmd

    e = np.asarray(e, dtype=np.float32)
    h = np.asarray(h, dtype=np.float32)
    n_chunks = e.shape[1]

    if n_chunks not in _prog_cache:
        _prog_cache[n_chunks] = _build(n_chunks)
    nc = _prog_cache[n_chunks]

    consts = _prep_consts(
        np.asarray(Wq, np.float32), np.asarray(bq, np.float32),
        np.asarray(Wk, np.float32), np.asarray(bk, np.float32),
        np.asarray(Wv, np.float32), np.asarray(bv, np.float32),
        np.asarray(Wo, np.float32), np.asarray(bo, np.float32),
        np.asarray(ln_g, np.float32), np.asarray(ln_b, np.float32))

    in_maps = []
    for b in range(B):
        m = dict(consts)
        import ml_dtypes
        m["e"] = np.ascontiguousarray(e[b].reshape(n_chunks, R, D))
        m["hbf"] = np.ascontiguousarray(h[b]).astype(ml_dtypes.bfloat16)
        in_maps.append(m)

    res = run_bass_kernel_spmd(nc, in_maps, core_ids=list(range(B)))
    out = np.stack([r["out"] for r in res.results], axis=0)
    return out.reshape(B, n_chunks, N, S, D)
